# revision 1
# baseline (speedup 1.0000x reference)
"""Trainium2 Bass kernel for a GPT-2 style transformer block.

Sharding (8 NeuronCores, SPMD-uniform program):
  - Tokens (B*S = 4096) sharded contiguously: core c owns tokens [512c, 512c+512).
    LayerNorms, QKV projection, attn out-proj, MLP all run on the local 512 tokens.
  - Attention is head-sharded: core c computes heads {2c, 2c+1} over ALL tokens.
    Two AllToAlls exchange (Q^T, K^T, V) token-shards -> head-shards, and the
    attention output O^T head-shards -> token-shards.
  - LayerNorm scale/bias are folded into the following matmul weights on host.
  - Matmuls run as float32r (full PE rate at free-dim >= 256); data stays fp32.
  - Softmax: scores are built transposed S^T[k, q] so exp() output A^T feeds the
    AV matmul directly (lhsT = [V | ones] augmented to also produce the softmax
    sums); normalization by 1/sum is applied on the O^T eviction.
"""

import numpy as np

# ---------------------------------------------------------------- config

B, S, D, H = 2, 2048, 1024, 16
HD = D // H           # 64
FF = 4 * D            # 4096
NC = 8                # cores
TPC = B * S // NC     # 512 tokens per core
EPS = 1e-05

P = 128               # partitions
TT = TPC // P         # 4 token tiles per core
DK = D // P           # 8 contraction tiles over D
FFK = FF // P         # 32 tiles over FF
HPC = H // NC         # 2 heads per core
QB = TPC              # q-block width for attention (= shard width)
NQB = S // QB         # 4 q-blocks per batch
KPB = QB // P         # 4 k-tiles per q-block


def build_program(debug_taps=False, reps=1):
    import contextlib

    import concourse.bass as bass
    import concourse.mybir as mybir
    import concourse.tile as tile
    from concourse import bacc
    from concourse.masks import make_identity, make_upper_triangular

    f32 = mybir.dt.float32
    f32r = mybir.dt.float32r
    AF = mybir.ActivationFunctionType

    nc = bacc.Bacc("TRN2", target_bir_lowering=False, debug=False,
                   num_devices=NC)

    # ---- kernel I/O (per core) ----
    x_d = nc.dram_tensor("x", [TPC, D], f32, kind="ExternalInput").ap()
    caw_d = nc.dram_tensor("c_attn_w", [D, 3 * D], f32r, kind="ExternalInput").ap()
    cab_d = nc.dram_tensor("c_attn_b", [3 * D], f32, kind="ExternalInput").ap()
    cpw_d = nc.dram_tensor("c_proj_w", [D, D], f32r, kind="ExternalInput").ap()
    cpb_d = nc.dram_tensor("c_proj_b", [D], f32, kind="ExternalInput").ap()
    fcw_d = nc.dram_tensor("fc_w", [D, FF], f32r, kind="ExternalInput").ap()
    fcb_d = nc.dram_tensor("fc_b", [FF], f32, kind="ExternalInput").ap()
    pjw_d = nc.dram_tensor("proj_w", [FF, D], f32r, kind="ExternalInput").ap()
    pjb_d = nc.dram_tensor("proj_b", [D], f32, kind="ExternalInput").ap()
    out_d = nc.dram_tensor("out", [TPC, D], f32, kind="ExternalOutput").ap()
    dbg = {}
    if debug_taps:
        dbg["fin"] = nc.dram_tensor("dbg_fin", [NC, 3 * P * TPC], f32r,
                                    kind="ExternalOutput").ap()
        dbg["fout"] = nc.dram_tensor("dbg_fout", [NC, 3 * P * TPC], f32r,
                                     kind="ExternalOutput").ap()
        dbg["bin"] = nc.dram_tensor("dbg_bin", [NC, P * TPC], f32r,
                                    kind="ExternalOutput").ap()
        dbg["bout"] = nc.dram_tensor("dbg_bout", [NC, P * TPC], f32r,
                                     kind="ExternalOutput").ap()

    SLOT = 3 * P * TPC

    with tile.TileContext(nc) as tc:
        ctx = contextlib.ExitStack()
        with ctx:
            dram = ctx.enter_context(tc.tile_pool(name="dram", bufs=1,
                                                  space="DRAM"))
            consts = ctx.enter_context(tc.tile_pool(name="consts", bufs=1))
            stats = ctx.enter_context(tc.tile_pool(name="stats", bufs=2))
            resid = ctx.enter_context(tc.tile_pool(name="resid", bufs=1))
            big = ctx.enter_context(tc.tile_pool(name="big", bufs=1))
            wpool = ctx.enter_context(tc.tile_pool(name="wpool", bufs=6))
            rhsp = ctx.enter_context(tc.tile_pool(name="rhsp", bufs=4))
            temps = ctx.enter_context(tc.tile_pool(name="temps", bufs=4))
            atp = ctx.enter_context(tc.tile_pool(name="atp", bufs=2))
            attin = ctx.enter_context(tc.tile_pool(name="attin", bufs=3))
            psum = ctx.enter_context(tc.tile_pool(name="psum", bufs=1,
                                                  space="PSUM"))

            # fwd slot j: [QT 128xTPC | KT 128xTPC | V TPCx128] for rank j heads
            a2a_fin = dram.tile([NC, SLOT], f32r)
            a2a_fout = dram.tile([NC, SLOT], f32r)
            a2a_bin = dram.tile([NC, P * TPC], f32r)
            a2a_bout = dram.tile([NC, P * TPC], f32r)

            def fwd_in_qt(j):
                return a2a_fin[j, 0:P * TPC].rearrange("(a b) -> a b", b=TPC)

            def fwd_in_kt(j):
                return a2a_fin[j, P * TPC:2 * P * TPC].rearrange(
                    "(a b) -> a b", b=TPC)

            def fwd_in_v(j):
                return a2a_fin[j, 2 * P * TPC:3 * P * TPC].rearrange(
                    "(a b) -> a b", b=P)

            def fwd_out_qt(j):
                return a2a_fout[j, 0:P * TPC].rearrange("(a b) -> a b", b=TPC)

            def fwd_out_kt(j):
                return a2a_fout[j, P * TPC:2 * P * TPC].rearrange(
                    "(a b) -> a b", b=TPC)

            def fwd_out_v(j):
                return a2a_fout[j, 2 * P * TPC:3 * P * TPC].rearrange(
                    "(a b) -> a b", b=P)

            # ---------------- constants ----------------
            ident = consts.tile([P, P], f32)
            make_identity(nc, ident)
            # mask[k, q] = 1 if q >= k else 0 (diagonal 128x128 strips)
            mask_f = consts.tile([P, P], f32)
            make_upper_triangular(nc, mask_f, val=1.0, diag=True)
            mask = consts.tile([P, P], f32r)
            nc.vector.tensor_copy(out=mask, in_=mask_f)
            # f32r-typed constants (memset cannot write f32r directly)
            ones_f = consts.tile([P, 1], f32)
            nc.vector.memset(ones_f, 1.0)
            ones_rr = consts.tile([P, 1], f32r)
            nc.vector.tensor_copy(out=ones_rr, in_=ones_f)
            zero_f = consts.tile([P, (KPB - 1) * P], f32)
            nc.vector.memset(zero_f, 0.0)
            zero_rr = consts.tile([P, (KPB - 1) * P], f32r)
            nc.vector.tensor_copy(out=zero_rr, in_=zero_f)

            cab_qk = consts.tile([P, 2 * DK], f32)   # c_attn_b[0:2D] as [P, 16]
            nc.sync.dma_start(cab_qk, cab_d[0:2 * D].rearrange("(m p) -> p m", p=P))
            fcb_sb = consts.tile([P, FFK], f32)      # fc_b as [P, 32]
            nc.sync.dma_start(fcb_sb, fcb_d.rearrange("(m p) -> p m", p=P))

            def bcast_row(src_ap, off, n):
                t = temps.tile([P, n], f32, tag="ln_out", name="bcast", bufs=2)
                nc.sync.dma_start(t, bass.AP(
                    tensor=src_ap.tensor, offset=src_ap.offset + off,
                    ap=[[0, P], [1, n]]))
                return t

            # ---------------- helpers ----------------
            def ln_transpose(src_tiles, dstT):
                """LayerNorm (ddof=1, eps on std, no scale/bias) each [P, D]
                token tile, then PE-transpose into dstT [P, DK, TPC]."""
                for t in range(TT):
                    xt = src_tiles[t]
                    st = stats.tile([P, 2, nc.vector.BN_STATS_DIM], f32,
                                    tag="bnst")
                    xg = xt.rearrange("p (g d) -> p g d", g=2)
                    for g in range(2):
                        nc.vector.bn_stats(out=st[:, g, :], in_=xg[:, g, :])
                    mv = stats.tile([P, nc.vector.BN_AGGR_DIM], f32, tag="mv")
                    nc.vector.bn_aggr(out=mv, in_=st)
                    sdev = stats.tile([P, 1], f32, tag="sdev")
                    nc.scalar.activation(out=sdev, in_=mv[:, 1:2], func=AF.Sqrt,
                                         scale=float(D) / (D - 1))
                    nc.vector.tensor_scalar_add(sdev, sdev, EPS)
                    rstd = stats.tile([P, 1], f32, tag="rstd")
                    nc.vector.reciprocal(out=rstd, in_=sdev)
                    nmr = stats.tile([P, 1], f32, tag="nmr")
                    nc.vector.tensor_scalar(out=nmr, in0=mv[:, 0:1],
                                            scalar1=rstd, scalar2=-1.0,
                                            op0=mybir.AluOpType.mult,
                                            op1=mybir.AluOpType.mult)
                    xn = temps.tile([P, D], f32, tag="ln_out", bufs=2)
                    nc.scalar.activation(out=xn, in_=xt, func=AF.Identity,
                                         bias=nmr, scale=rstd)
                    for d in range(DK):
                        pt = psum.tile([P, P], f32, tag="tp", bufs=2)
                        nc.tensor.transpose(pt, xn[:, d * P:(d + 1) * P], ident)
                        nc.vector.tensor_copy(out=dstT[:, d, t * P:(t + 1) * P],
                                              in_=pt)

            def whole_block():
                # ---------------- phase 1: load x, LN1 + transpose ----------------
                x_tiles = [resid.tile([P, D], f32, tag=f"x{t}", name=f"x{t}")
                           for t in range(TT)]
                xr = x_d.rearrange("(t p) d -> t p d", p=P)
                for t in range(TT):
                    nc.sync.dma_start(x_tiles[t], xr[t])

                xnT = big.tile([P, DK, TPC], f32r, tag="xT")
                ln_transpose(x_tiles, xnT)

                # ---------------- phase 2: QKV projections ----------------
                for m in range(2 * DK):  # 16 feature tiles: Q then K
                    ps = psum.tile([P, TPC], f32, tag="mmps", bufs=2)
                    for k in range(DK):
                        wt = wpool.tile([P, P], f32r, tag="wlhs")
                        nc.sync.dma_start(wt, caw_d[k * P:(k + 1) * P,
                                                    m * P:(m + 1) * P])
                        nc.tensor.matmul(ps, wt,
                                         xnT[:, k, :],
                                         start=(k == 0), stop=(k == DK - 1))
                    sb = temps.tile([P, TPC], f32r, tag="ev512")
                    nc.scalar.activation(out=sb, in_=ps, func=AF.Identity,
                                         bias=cab_qk[:, m:m + 1])
                    j, half = m % DK, m // DK
                    dst = fwd_in_qt(j) if half == 0 else fwd_in_kt(j)
                    nc.sync.dma_start(dst, sb)

                # V token-major
                vb_bc = bcast_row(cab_d, 2 * D, D)
                for t in range(TT):
                    for nb in range(2):
                        ns = D // 2
                        ps = psum.tile([P, ns], f32, tag="mmps", bufs=2)
                        for k in range(DK):
                            wt = rhsp.tile([P, ns], f32r, tag="wrhs")
                            nc.sync.dma_start(wt, caw_d[k * P:(k + 1) * P,
                                                        2 * D + nb * ns:
                                                        2 * D + (nb + 1) * ns])
                            nc.tensor.matmul(
                                ps, xnT[:, k, t * P:(t + 1) * P],
                                wt,
                                start=(k == 0), stop=(k == DK - 1))
                        sb = temps.tile([P, ns], f32r, tag="ev512")
                        nc.vector.tensor_add(out=sb, in0=ps,
                                             in1=vb_bc[:, nb * ns:(nb + 1) * ns])
                        for jj in range(4):
                            j = nb * 4 + jj
                            nc.sync.dma_start(fwd_in_v(j)[t * P:(t + 1) * P, :],
                                              sb[:, jj * P:(jj + 1) * P])

                # ---------------- phase 3: forward AllToAll ----------------
                if debug_taps:
                    nc.sync.dma_start(dbg["fin"][:], a2a_fin[:])
                nc.gpsimd.collective_compute(
                    "AllToAll", mybir.AluOpType.bypass,
                    replica_groups=[list(range(NC))],
                    ins=[a2a_fin.opt()], outs=[a2a_fout.opt()])
                if debug_taps:
                    nc.sync.dma_start(dbg["fout"][:], a2a_fout[:])

                # ---------------- phase 4: attention (my 2 heads, all tokens) ----
                for b in range(B):
                    for qb in range(NQB):
                        slot_q = b * NQB + qb
                        qt_sb = attin.tile([P, QB], f32r, tag="qt")
                        nc.sync.dma_start(qt_sb, fwd_out_qt(slot_q))
                        opsums = [psum.tile([HD + 1, QB], f32, tag=f"op{h}",
                                            name=f"op{h}", bufs=1)
                                  for h in range(HPC)]
                        nkt = (qb + 1) * KPB
                        for kt in range(nkt):
                            slot_k = b * NQB + kt // KPB
                            off = (kt % KPB) * P
                            kt_sb = attin.tile([P, P], f32r, tag="kt")
                            nc.sync.dma_start(kt_sb,
                                              fwd_out_kt(slot_k)[:, off:off + P])
                            va = attin.tile([P, HPC, HD + 1], f32r, tag="va")
                            nc.sync.dma_start(
                                va[:, :, 0:HD],
                                fwd_out_v(slot_k)[off:off + P, :].rearrange(
                                    "p (h d) -> p h d", h=HPC))
                            for h in range(HPC):
                                nc.vector.tensor_copy(out=va[:, h, HD:HD + 1],
                                                      in_=ones_rr)
                            d = kt - qb * KPB  # >= 0 on diagonal strips
                            for h in range(HPC):
                                sps = psum.tile([P, QB], f32, tag=f"s{h}", bufs=1)
                                nc.tensor.matmul(
                                    sps, kt_sb[h * HD:(h + 1) * HD, :],
                                    qt_sb[h * HD:(h + 1) * HD, :],
                                    start=True, stop=True)
                                at = atp.tile([P, QB], f32r, tag=f"at{h}")
                                if d >= 0:
                                    if d > 0:
                                        nc.vector.tensor_copy(out=at[:, 0:d * P],
                                                              in_=zero_rr[:, 0:d * P])
                                    nc.scalar.activation(
                                        out=at[:, d * P:], in_=sps[:, d * P:],
                                        func=AF.Exp,
                                        scale=1.0 / float(np.sqrt(HD)))
                                    nc.vector.tensor_mul(
                                        out=at[:, d * P:(d + 1) * P],
                                        in0=at[:, d * P:(d + 1) * P], in1=mask)
                                else:
                                    nc.scalar.activation(
                                        out=at, in_=sps, func=AF.Exp,
                                        scale=1.0 / float(np.sqrt(HD)))
                                nc.tensor.matmul(opsums[h],
                                                 va[:, h, :],
                                                 at,
                                                 start=(kt == 0),
                                                 stop=(kt == nkt - 1))
                        # normalize and ship O^T shard to its token-owner rank
                        for h in range(HPC):
                            rs = stats.tile([P, QB], f32, tag="rs")
                            nc.vector.reciprocal(out=rs[HD:HD + 1, :],
                                                 in_=opsums[h][HD:HD + 1, :])
                            rsd = dram.tile([QB], f32, tag="rsd", name="rsd",
                                            bufs=2)
                            nc.sync.dma_start(rsd, rs[HD:HD + 1, :])
                            rbc = stats.tile([HD, QB], f32, tag="rbc")
                            nc.sync.dma_start(rbc, bass.AP(
                                tensor=rsd.tensor, offset=rsd.offset,
                                ap=[[0, HD], [1, QB]]))
                            otv = temps.tile([HD, QB], f32r, tag="ev512")
                            nc.vector.tensor_mul(out=otv, in0=opsums[h][0:HD, :],
                                                 in1=rbc)
                            nc.sync.dma_start(
                                a2a_bin[slot_q,
                                        h * HD * TPC:(h + 1) * HD * TPC].rearrange(
                                            "(a b) -> a b", b=TPC),
                                otv)

                if debug_taps:
                    nc.sync.dma_start(dbg["bin"][:], a2a_bin[:])
                nc.gpsimd.collective_compute(
                    "AllToAll", mybir.AluOpType.bypass,
                    replica_groups=[list(range(NC))],
                    ins=[a2a_bin.opt()], outs=[a2a_bout.opt()])
                if debug_taps:
                    nc.sync.dma_start(dbg["bout"][:], a2a_bout[:])

                # ---------------- phase 5: attn out-proj + residual ----------------
                cpb_bc = bcast_row(cpb_d, 0, D)
                otf = big.tile([P, DK, TPC], f32r, tag="otf")
                nc.sync.dma_start(otf, a2a_bout[:].rearrange(
                    "n (p t) -> p n t", p=P))
                x2_tiles = [resid.tile([P, D], f32, tag=f"x2{t}", name=f"x2{t}")
                            for t in range(TT)]
                for t in range(TT):
                    for nb in range(2):
                        ns = D // 2
                        ps = psum.tile([P, ns], f32, tag="mmps", bufs=2)
                        for k in range(DK):
                            wt = rhsp.tile([P, ns], f32r, tag="wrhs")
                            nc.sync.dma_start(wt, cpw_d[k * P:(k + 1) * P,
                                                        nb * ns:(nb + 1) * ns])
                            nc.tensor.matmul(
                                ps, otf[:, k, t * P:(t + 1) * P],
                                wt,
                                start=(k == 0), stop=(k == DK - 1))
                        sl = slice(nb * ns, (nb + 1) * ns)
                        nc.vector.tensor_add(out=x2_tiles[t][:, sl], in0=ps,
                                             in1=x_tiles[t][:, sl])
                        nc.vector.tensor_add(out=x2_tiles[t][:, sl],
                                             in0=x2_tiles[t][:, sl],
                                             in1=cpb_bc[:, sl])

                # ---------------- phase 6: LN2 + transpose ----------------
                xn2T = big.tile([P, DK, TPC], f32r, tag="xT")
                ln_transpose(x2_tiles, xn2T)

                # ---------------- phase 7: MLP fc + gelu ----------------
                # tanh-approx gelu, exact form: gelu(u) = u * sigmoid(g(u)),
                # g(u) = 2*sqrt(2/pi) * (u + 0.044715 u^3) = K1*u + K2*u^3
                K1 = 2.0 * float(np.sqrt(2.0 / np.pi))
                K2 = K1 * 0.044715
                hT = big.tile([P, FFK, TPC], f32r, tag="hT")
                for m in range(FFK):
                    ps = psum.tile([P, TPC], f32, tag="mmps", bufs=2)
                    for k in range(DK):
                        wt = wpool.tile([P, P], f32r, tag="wlhs")
                        nc.sync.dma_start(wt, fcw_d[k * P:(k + 1) * P,
                                                    m * P:(m + 1) * P])
                        nc.tensor.matmul(ps, wt,
                                         xn2T[:, k, :],
                                         start=(k == 0), stop=(k == DK - 1))
                    u = temps.tile([P, TPC], f32, tag="ev512")
                    nc.scalar.activation(out=u, in_=ps, func=AF.Identity,
                                         bias=fcb_sb[:, m:m + 1])
                    g = temps.tile([P, TPC], f32, tag="ev512")
                    nc.scalar.activation(out=g, in_=u, func=AF.Square,
                                         scale=float(np.sqrt(K2)))
                    nc.vector.tensor_scalar_add(g, g, K1)
                    nc.vector.tensor_mul(out=g, in0=g, in1=u)
                    nc.scalar.activation(out=g, in_=g, func=AF.Sigmoid)
                    nc.vector.tensor_mul(out=hT[:, m, :], in0=g, in1=u)

                # ---------------- phase 8: MLP proj + residual -> out ----------------
                pjb_bc = bcast_row(pjb_d, 0, D)
                outr = out_d.rearrange("(t p) d -> t p d", p=P)
                for t in range(TT):
                    ob = temps.tile([P, D], f32, tag="ln_out", bufs=2)
                    for nb in range(2):
                        ns = D // 2
                        ps = psum.tile([P, ns], f32, tag="mmps", bufs=2)
                        for k in range(FFK):
                            wt = rhsp.tile([P, ns], f32r, tag="wrhs")
                            nc.sync.dma_start(wt, pjw_d[k * P:(k + 1) * P,
                                                        nb * ns:(nb + 1) * ns])
                            nc.tensor.matmul(
                                ps, hT[:, k, t * P:(t + 1) * P],
                                wt,
                                start=(k == 0), stop=(k == FFK - 1))
                        sl = slice(nb * ns, (nb + 1) * ns)
                        nc.vector.tensor_add(out=ob[:, sl], in0=ps,
                                             in1=x2_tiles[t][:, sl])
                        nc.vector.tensor_add(out=ob[:, sl], in0=ob[:, sl],
                                             in1=pjb_bc[:, sl])
                    nc.sync.dma_start(outr[t], ob)


            for _rep in range(reps):
                whole_block()

    nc.compile()
    return nc


_NC_CACHE = None


def _get_program():
    global _NC_CACHE
    if _NC_CACHE is None:
        _NC_CACHE = build_program()
    return _NC_CACHE


def host_fold(inputs):
    """Fold LN scale/bias into the following matmul weights (host side)."""
    def f(a):
        return np.ascontiguousarray(np.asarray(a), dtype=np.float32)
    x = f(inputs["x"]).reshape(B * S, D)
    caw0 = f(inputs["c_attn_w"])
    fcw0 = f(inputs["fc_w"])
    caw = caw0 * f(inputs["ln1_w"])[:, None]
    cab = f(inputs["c_attn_b"]) + f(inputs["ln1_b"]) @ caw0
    fcw = fcw0 * f(inputs["ln2_w"])[:, None]
    fcb = f(inputs["fc_b"]) + f(inputs["ln2_b"]) @ fcw0
    return {
        "x": x,
        "c_attn_w": f(caw), "c_attn_b": f(cab),
        "c_proj_w": f(inputs["c_proj_w"]), "c_proj_b": f(inputs["c_proj_b"]),
        "fc_w": f(fcw), "fc_b": f(fcb),
        "proj_w": f(inputs["proj_w"]), "proj_b": f(inputs["proj_b"]),
    }


def make_in_maps(inputs):
    full = host_fold(inputs)
    in_maps = []
    for c in range(NC):
        m = dict(full)
        m["x"] = np.ascontiguousarray(full["x"][c * TPC:(c + 1) * TPC])
        in_maps.append(m)
    return in_maps


def kernel(**inputs) -> np.ndarray:
    from concourse import bass_utils
    nc = _get_program()
    in_maps = make_in_maps(inputs)
    res = bass_utils.run_bass_kernel_spmd(nc, in_maps, core_ids=list(range(NC)))
    out = np.concatenate([res.results[c]["out"] for c in range(NC)], axis=0)
    return out.reshape(B, S, D)



# revision 9
# speedup vs baseline: 2.2737x; 2.2737x over previous
"""Trainium2 Bass kernel for a GPT-2 style transformer block (v2, bf16).

Sharding (8 NeuronCores, SPMD-uniform program):
  - Tokens (B*S = 4096) sharded contiguously: core c owns tokens [512c, 512c+512).
  - Attention is head-sharded: core c computes heads {2c, 2c+1} over ALL tokens.
    AllToAlls exchange (Q^T, K^T) and V token-shards -> head-shards (split in two
    collectives so V compute overlaps the QK exchange), and O^T back.
  - All matmul operands are bf16 (fp32 PSUM accumulate); the residual stream
    stays fp32 in SBUF. LN scale/bias folded into following weights on host.
  - Weights live in SBUF slabs rotating through one shared pool tag so DMA
    prefetch of later phases overlaps earlier compute.
  - Attention K/V are SBUF-resident; scores are built transposed S^T[k, q], the
    exp output A^T feeds AV directly; lhsT = [ones | V_h] also produces softmax
    sums; normalization uses a PE broadcast matmul (no DRAM round-trip), with
    causal trimming of score/exp/AV free dims.
  - MLP runs in 8 fused blocks: fc (weight-stationary) -> gelu (one scalar
    activation instr, tanh approx) -> proj (h-stationary) accumulated into the
    fp32 residual in SBUF.
"""

import numpy as np

# ---------------------------------------------------------------- config

B, S, D, H = 2, 2048, 1024, 16
HD = D // H           # 64
FF = 4 * D            # 4096
NC = 8                # cores
TPC = B * S // NC     # 512 tokens per core
EPS = 1e-05

P = 128               # partitions
TT = TPC // P         # 4 token tiles per core
DK = D // P           # 8 contraction tiles over D
FFK = FF // P         # 32 tiles over FF
HPC = H // NC         # 2 heads per core
QB = TPC              # q-block width for attention (= shard width)
NQB = S // QB         # 4 q-blocks per batch
KPB = QB // P         # 4 k-tiles per q-block
NBLK = 8              # fused fc/proj blocks (512 ff features each)


def build_program():
    import contextlib

    import concourse.bass as bass
    import concourse.mybir as mybir
    import concourse.tile as tile
    from concourse import bacc
    from concourse.masks import make_identity, make_upper_triangular

    f32 = mybir.dt.float32
    f32r = mybir.dt.float32r
    bf16 = mybir.dt.bfloat16
    AF = mybir.ActivationFunctionType

    nc = bacc.Bacc("TRN2", target_bir_lowering=False, debug=False,
                   num_devices=NC)

    # ---- kernel I/O (per core) ----
    x_d = nc.dram_tensor("x", [TPC, D], f32, kind="ExternalInput").ap()
    caw_d = nc.dram_tensor("c_attn_w", [D, 3 * D], bf16, kind="ExternalInput").ap()
    cab_d = nc.dram_tensor("c_attn_b", [3 * D], f32, kind="ExternalInput").ap()
    cpw_d = nc.dram_tensor("c_proj_w", [D, D], bf16, kind="ExternalInput").ap()
    cpb_d = nc.dram_tensor("c_proj_b", [D], f32, kind="ExternalInput").ap()
    fcw_d = nc.dram_tensor("fc_w", [D, FF], bf16, kind="ExternalInput").ap()
    fcb_d = nc.dram_tensor("fc_b", [FF], f32, kind="ExternalInput").ap()
    pjw_d = nc.dram_tensor("proj_w", [FF, D], bf16, kind="ExternalInput").ap()
    pjb_d = nc.dram_tensor("proj_b", [D], f32, kind="ExternalInput").ap()
    out_d = nc.dram_tensor("out", [TPC, D], f32, kind="ExternalOutput").ap()

    PT = P * TPC  # elements in one [128, 512] slot region

    with tile.TileContext(nc) as tc:
        ctx = contextlib.ExitStack()
        with ctx:
            dram = ctx.enter_context(tc.tile_pool(name="dram", bufs=1,
                                                  space="DRAM"))
            consts = ctx.enter_context(tc.tile_pool(name="consts", bufs=1))
            wsl = ctx.enter_context(tc.tile_pool(name="wsl", bufs=13))
            resid = ctx.enter_context(tc.tile_pool(name="resid", bufs=1))
            acts = ctx.enter_context(tc.tile_pool(name="acts", bufs=1))
            attp = ctx.enter_context(tc.tile_pool(name="attp", bufs=1))
            temps = ctx.enter_context(tc.tile_pool(name="temps", bufs=4))
            stats = ctx.enter_context(tc.tile_pool(name="stats", bufs=2))
            psum = ctx.enter_context(tc.tile_pool(name="psum", bufs=1,
                                                  space="PSUM"))

            # a2a buffers (bf16)
            finQK = dram.tile([NC, 2 * PT], bf16)
            foutQK = dram.tile([NC, 2 * PT], bf16)
            finV = dram.tile([NC, PT], bf16)
            foutV = dram.tile([NC, PT], bf16)
            a2a_bin = dram.tile([NC, PT], bf16)
            a2a_bout = dram.tile([NC, PT], bf16)

            # ---------------- constants ----------------
            ident = consts.tile([P, P], bf16)
            make_identity(nc, ident)
            # mask[k, q] = 1 if q >= k (within a diagonal 128x128 strip)
            mask_f = consts.tile([P, P], f32)
            make_upper_triangular(nc, mask_f, val=1.0, diag=True)
            mask2 = consts.tile([P, HPC, P], bf16)
            for h in range(HPC):
                nc.vector.tensor_copy(out=mask2[:, h, :], in_=mask_f)

            ones_f = consts.tile([1, HD], f32)
            nc.vector.memset(ones_f, 1.0)
            ones64 = consts.tile([1, HD], f32r)
            nc.vector.tensor_copy(out=ones64, in_=ones_f)

            cab_qk = consts.tile([P, 2 * DK], f32)   # c_attn_b[0:2D] as [P, 16]
            nc.sync.dma_start(cab_qk, cab_d[0:2 * D].rearrange("(m p) -> p m", p=P))
            fcb_pp = consts.tile([P, FFK], f32)      # fc_b as [P, 32]
            nc.sync.dma_start(fcb_pp, fcb_d.rearrange("(m p) -> p m", p=P))

            def bcast_row(src_ap, off, n, name):
                t = consts.tile([P, n], f32, name=name)
                nc.sync.dma_start(t, bass.AP(
                    tensor=src_ap.tensor, offset=src_ap.offset + off,
                    ap=[[0, P], [1, n]]))
                return t

            vb_bc = bcast_row(cab_d, 2 * D, D, "vb_bc")    # V bias along feats
            cpb_bc = bcast_row(cpb_d, 0, D, "cpb_bc")
            pjb_bc = bcast_row(pjb_d, 0, D, "pjb_bc")

            # ---------------- weight slabs (shared rotating tag) ----------------
            # caw: 8 slabs [128, 3072]; cpw: 2 slabs [128, 4, 1024];
            # fcw: 8 m-slabs [128, 8, 512]; pjw: 8 slabs [128, 4, 1024].
            caw_r = caw_d.rearrange("(a p) f -> p a f", p=P)   # [128, 8, 3072]
            cpw_r = cpw_d.rearrange("(a p) d -> p a d", p=P)   # [128, 8, 1024]
            fcw_r = fcw_d.rearrange("(a p) f -> p a f", p=P)   # [128, 8, 4096]
            pjw_r = pjw_d.rearrange("(a p) d -> p a d", p=P)   # [128, 32, 1024]

            caw_sb = []
            for k in range(DK):
                t = wsl.tile([P, 3 * D], bf16, tag="wsl", name=f"caw{k}")
                nc.gpsimd.dma_start(t, caw_r[:, k, :])
                caw_sb.append(t)
            cpw_sb = []
            for c in range(2):
                t = wsl.tile([P, 4, D], bf16, tag="wsl", name=f"cpw{c}")
                nc.gpsimd.dma_start(t, cpw_r[:, 4 * c:4 * c + 4, :])
                cpw_sb.append(t)
            fcw_sb = []
            for g in range(NBLK):
                t = wsl.tile([P, DK, 512], bf16, tag="wsl", name=f"fcw{g}")
                nc.gpsimd.dma_start(t, fcw_r[:, :, g * 512:(g + 1) * 512])
                fcw_sb.append(t)
            # pjw slabs are allocated later (scalar queue) to avoid queue
            # inversion with the collectives on gpsimd.

            # ---------------- phase 1: load x, LN1 + transpose ----------------
            x_tiles = [resid.tile([P, D], f32, tag=f"x{t}", name=f"x{t}")
                       for t in range(TT)]
            xr = x_d.rearrange("(t p) d -> t p d", p=P)
            for t in range(TT):
                nc.sync.dma_start(x_tiles[t], xr[t])

            def ln_transpose(dstT):
                """LayerNorm (ddof=1, eps on std) each [P, D] token tile of the
                residual, then PE-transpose into dstT [P, DK, TPC] (bf16)."""
                for t in range(TT):
                    xt = x_tiles[t]
                    st = stats.tile([P, 2, nc.vector.BN_STATS_DIM], f32,
                                    tag="bnst")
                    xg = xt.rearrange("p (g d) -> p g d", g=2)
                    for g in range(2):
                        nc.vector.bn_stats(out=st[:, g, :], in_=xg[:, g, :])
                    mv = stats.tile([P, nc.vector.BN_AGGR_DIM], f32, tag="mv")
                    nc.vector.bn_aggr(out=mv, in_=st)
                    sdev = stats.tile([P, 1], f32, tag="sdev")
                    nc.scalar.activation(out=sdev, in_=mv[:, 1:2], func=AF.Sqrt,
                                         scale=float(D) / (D - 1))
                    nc.vector.tensor_scalar_add(sdev, sdev, EPS)
                    rstd = stats.tile([P, 1], f32, tag="rstd")
                    nc.vector.reciprocal(out=rstd, in_=sdev)
                    nmr = stats.tile([P, 1], f32, tag="nmr")
                    nc.vector.tensor_scalar(out=nmr, in0=mv[:, 0:1],
                                            scalar1=rstd, scalar2=-1.0,
                                            op0=mybir.AluOpType.mult,
                                            op1=mybir.AluOpType.mult)
                    xn = temps.tile([P, D], bf16, tag="xn", bufs=2)
                    nc.scalar.activation(out=xn, in_=xt, func=AF.Identity,
                                         bias=nmr, scale=rstd)
                    for d in range(DK):
                        pt = psum.tile([P, P], bf16, tag="ps2", bufs=2,
                                       name="tp")
                        nc.tensor.transpose(pt, xn[:, d * P:(d + 1) * P], ident)
                        nc.vector.tensor_copy(out=dstT[:, d, t * P:(t + 1) * P],
                                              in_=pt)

            xnT = acts.tile([P, DK, TPC], bf16, tag="xnT", name="xnT")
            ln_transpose(xnT)

            # ---------------- phase 2a: Q,K projections -> finQK ----------------
            for m in range(2 * DK):  # 16 feature tiles: Q then K
                ps = psum.tile([P, TPC], f32, tag="ps1", bufs=2, name="mmps")
                for k in range(DK):
                    nc.tensor.matmul(ps, caw_sb[k][:, m * P:(m + 1) * P],
                                     xnT[:, k, :],
                                     start=(k == 0), stop=(k == DK - 1))
                sb = temps.tile([P, TPC], bf16, tag="ev", bufs=3, name="ev")
                nc.scalar.activation(out=sb, in_=ps, func=AF.Identity,
                                     bias=cab_qk[:, m:m + 1])
                j, half = m % DK, m // DK
                dst = finQK[j, half * PT:(half + 1) * PT].rearrange(
                    "(a b) -> a b", b=TPC)
                nc.sync.dma_start(dst, sb)

            # ---------------- phase 3a: forward AllToAll (Q,K) ----------------
            nc.gpsimd.collective_compute(
                "AllToAll", mybir.AluOpType.bypass,
                replica_groups=[list(range(NC))],
                ins=[finQK.opt()], outs=[foutQK.opt()])

            # ---------------- phase 2b: V projection -> finV ----------------
            for t in range(TT):
                for nb in range(2):
                    ns = D // 2
                    ps = psum.tile([P, ns], f32, tag="ps1", bufs=2, name="mmps")
                    for k in range(DK):
                        nc.tensor.matmul(
                            ps, xnT[:, k, t * P:(t + 1) * P],
                            caw_sb[k][:, 2 * D + nb * ns:2 * D + (nb + 1) * ns],
                            start=(k == 0), stop=(k == DK - 1))
                    vt = temps.tile([P, ns], bf16, tag="vt", bufs=2, name="vt")
                    nc.vector.tensor_add(out=vt, in0=ps,
                                         in1=vb_bc[:, nb * ns:(nb + 1) * ns])
                    for jj in range(4):
                        j = nb * 4 + jj
                        nc.sync.dma_start(
                            finV[j, t * P * P:(t + 1) * P * P].rearrange(
                                "(p f) -> p f", f=P),
                            vt[:, jj * P:(jj + 1) * P])

            # ---------------- phase 3b: forward AllToAll (V) ----------------
            nc.gpsimd.collective_compute(
                "AllToAll", mybir.AluOpType.bypass,
                replica_groups=[list(range(NC))],
                ins=[finV.opt()], outs=[foutV.opt()])

            # ---------------- phase 4: attention (my 2 heads, all tokens) ----
            # K^T resident: [128 feat, slot, 512 tok]
            ktr = attp.tile([P, NC, TPC], bf16, tag="ktr", name="ktr")
            nc.sync.dma_start(
                ktr, foutQK.rearrange("n (r p t) -> r p n t", r=2, p=P)[1])
            # V resident with ones col per head: [128 tok, slot, t, 2*(1+64)]
            vres = attp.tile([P, NC, TT, HPC * (HD + 1)], bf16, tag="vres",
                             name="vres")
            for j in range(NC):
                nc.vector.memset(
                    vres[:, j, :, :].rearrange("p t (h c) -> p t h c",
                                               c=HD + 1)[:, :, :, HD:HD + 1],
                    1.0)
                for t in range(TT):
                    dst = vres[:, j, t, :].rearrange(
                        "p (h c) -> p h c", c=HD + 1)[:, :, 0:HD]
                    src = foutV[j, t * P * P:(t + 1) * P * P].rearrange(
                        "(p h c) -> p h c", p=P, h=HPC)
                    nc.sync.dma_start(dst, src)

            isc = 1.0 / float(np.sqrt(HD))
            for b in range(B):
                for qb in range(NQB):
                    slot_q = b * NQB + qb
                    qt = attp.tile([P, QB], bf16, tag="qt", bufs=2, name="qt")
                    nc.sync.dma_start(
                        qt, foutQK[slot_q, 0:PT].rearrange("(a b) -> a b",
                                                           b=TPC))
                    op = psum.tile([HD + 1, HPC * QB], f32, tag="op", bufs=1,
                                   name="op")
                    nkt = (qb + 1) * KPB
                    for kt in range(nkt):
                        src_slot = b * NQB + kt // KPB
                        off = (kt % KPB) * P
                        d = kt - qb * KPB  # >= 0 on diagonal strips
                        qs = max(d, 0) * P
                        sps = psum.tile([P, HPC * QB], f32, tag="ps2", bufs=2,
                                        name="sps")
                        for h in range(HPC):
                            nc.tensor.matmul(
                                sps[:, h * QB + qs:(h + 1) * QB],
                                ktr[h * HD:(h + 1) * HD, src_slot,
                                    off:off + P],
                                qt[h * HD:(h + 1) * HD, qs:QB],
                                start=True, stop=True)
                        at = attp.tile([P, HPC, QB], bf16, tag="at", bufs=3,
                                       name="at")
                        spsv = sps.rearrange("p (h q) -> p h q", h=HPC)
                        nc.scalar.activation(out=at[:, :, qs:QB],
                                             in_=spsv[:, :, qs:QB],
                                             func=AF.Exp, scale=isc)
                        if d >= 0:
                            nc.vector.tensor_mul(out=at[:, :, qs:qs + P],
                                                 in0=at[:, :, qs:qs + P],
                                                 in1=mask2)
                        for h in range(HPC):
                            nc.tensor.matmul(
                                op[:, h * QB + qs:(h + 1) * QB],
                                vres[:, src_slot, kt % KPB,
                                     h * (HD + 1):(h + 1) * (HD + 1)],
                                at[:, h, qs:QB],
                                start=(kt == 0), stop=(kt == nkt - 1))
                    # normalize: recip of sums row, broadcast via PE matmul
                    rc = attp.tile([1, HPC * QB], f32r, tag="rc", bufs=2,
                                   name="rc")
                    with nc.allow_low_precision(reason="f32r == f32 bitwise"):
                        nc.vector.reciprocal(out=rc, in_=op[HD:HD + 1, :])
                    otu = attp.tile([HD, HPC * QB], bf16, tag="otu", bufs=2,
                                    name="otu")
                    nc.vector.tensor_copy(out=otu, in_=op[0:HD, :])
                    otv = attp.tile([HD, HPC * QB], bf16, tag="otv", bufs=2,
                                    name="otv")
                    for h in range(HPC):
                        bc = psum.tile([HD, QB], f32, tag="ps1", bufs=2,
                                       name="bc")
                        nc.tensor.matmul(bc, ones64,
                                         rc[:, h * QB:(h + 1) * QB],
                                         start=True, stop=True)
                        nc.vector.tensor_mul(
                            out=otv[:, h * QB:(h + 1) * QB],
                            in0=otu[:, h * QB:(h + 1) * QB], in1=bc)
                        nc.sync.dma_start(
                            a2a_bin[slot_q, h * HD * TPC:(h + 1) * HD * TPC]
                            .rearrange("(a b) -> a b", b=TPC),
                            otv[:, h * QB:(h + 1) * QB])

            # ---------------- phase 3c: backward AllToAll (O^T) ----------------
            nc.gpsimd.collective_compute(
                "AllToAll", mybir.AluOpType.bypass,
                replica_groups=[list(range(NC))],
                ins=[a2a_bin.opt()], outs=[a2a_bout.opt()])

            # ---------------- phase 5: attn out-proj + residual (in place) ----
            otf = acts.tile([P, DK, TPC], bf16, tag="xnT", name="otf")
            nc.sync.dma_start(otf, a2a_bout[:].rearrange(
                "n (p t) -> p n t", p=P))
            # pjw slabs on the sync queue here: the only sync work emitted
            # after them is the final out DMA, so their slot-waits (on caw/cpw/
            # fcw slab releases, all PE-driven) cannot head-of-line block any
            # compute. Slabs 0-4 bind to slots already free by this point;
            # 5-7 bind as the first fc blocks release their fcw slabs.
            pjw_sb = []
            for g in range(NBLK):
                t = wsl.tile([P, 4, D], bf16, tag="wsl", name=f"pjw{g}")
                nc.sync.dma_start(t, pjw_r[:, 4 * g:4 * g + 4, :])
                pjw_sb.append(t)
            for t in range(TT):
                nc.vector.tensor_add(out=x_tiles[t], in0=x_tiles[t],
                                     in1=cpb_bc)
            for t in range(TT):
                for nb in range(2):
                    ns = D // 2
                    ps = psum.tile([P, ns], f32, tag="ps1", bufs=2, name="mmps")
                    for k in range(DK):
                        nc.tensor.matmul(
                            ps, otf[:, k, t * P:(t + 1) * P],
                            cpw_sb[k // 4][:, k % 4, nb * ns:(nb + 1) * ns],
                            start=(k == 0), stop=(k == DK - 1))
                    sl = slice(nb * ns, (nb + 1) * ns)
                    nc.vector.tensor_add(out=x_tiles[t][:, sl], in0=ps,
                                         in1=x_tiles[t][:, sl])

            # ---------------- phase 6: LN2 + transpose ----------------
            xn2T = acts.tile([P, DK, TPC], bf16, tag="xnT", name="xn2T")
            ln_transpose(xn2T)

            # ---------------- phase 7+8: fused MLP blocks ----------------
            for t in range(TT):
                nc.vector.tensor_add(out=x_tiles[t], in0=x_tiles[t],
                                     in1=pjb_bc)
            for g in range(NBLK):
                ht = []
                for mm in range(4):
                    m = g * 4 + mm
                    ps = psum.tile([P, TPC], f32, tag="ps1", bufs=2,
                                   name="mmps")
                    for k in range(DK):
                        nc.tensor.matmul(
                            ps, fcw_sb[g][:, k, mm * P:(mm + 1) * P],
                            xn2T[:, k, :],
                            start=(k == 0), stop=(k == DK - 1))
                    hm = temps.tile([P, TPC], bf16, tag="hT", bufs=8,
                                    name="hT")
                    nc.scalar.activation(out=hm, in_=ps,
                                         func=AF.Gelu_apprx_tanh,
                                         bias=fcb_pp[:, m:m + 1])
                    ht.append(hm)
                for t in range(TT):
                    for nb in range(2):
                        ns = D // 2
                        ps = psum.tile([P, ns], f32, tag="ps1", bufs=2,
                                       name="mmps")
                        for kk in range(4):
                            nc.tensor.matmul(
                                ps, ht[kk][:, t * P:(t + 1) * P],
                                pjw_sb[g][:, kk, nb * ns:(nb + 1) * ns],
                                start=(kk == 0), stop=(kk == 3))
                        sl = slice(nb * ns, (nb + 1) * ns)
                        nc.vector.tensor_add(out=x_tiles[t][:, sl], in0=ps,
                                             in1=x_tiles[t][:, sl])

            # ---------------- output ----------------
            outr = out_d.rearrange("(t p) d -> t p d", p=P)
            for t in range(TT):
                nc.sync.dma_start(outr[t], x_tiles[t])

    nc.compile()
    return nc


_NC_CACHE = None


def _get_program():
    global _NC_CACHE
    if _NC_CACHE is None:
        _NC_CACHE = build_program()
    return _NC_CACHE


def host_fold(inputs):
    """Fold LN scale/bias into the following matmul weights; cast weights to
    bf16 (host side)."""
    import ml_dtypes
    bf = ml_dtypes.bfloat16

    def f(a):
        return np.ascontiguousarray(np.asarray(a), dtype=np.float32)
    x = f(inputs["x"]).reshape(B * S, D)
    caw0 = f(inputs["c_attn_w"])
    fcw0 = f(inputs["fc_w"])
    caw = caw0 * f(inputs["ln1_w"])[:, None]
    cab = f(inputs["c_attn_b"]) + f(inputs["ln1_b"]) @ caw0
    fcw = fcw0 * f(inputs["ln2_w"])[:, None]
    fcb = f(inputs["fc_b"]) + f(inputs["ln2_b"]) @ fcw0

    def w(a):
        return np.ascontiguousarray(np.asarray(a, dtype=bf))
    return {
        "x": x,
        "c_attn_w": w(caw), "c_attn_b": f(cab),
        "c_proj_w": w(inputs["c_proj_w"]), "c_proj_b": f(inputs["c_proj_b"]),
        "fc_w": w(fcw), "fc_b": f(fcb),
        "proj_w": w(inputs["proj_w"]), "proj_b": f(inputs["proj_b"]),
    }


def make_in_maps(inputs):
    full = host_fold(inputs)
    in_maps = []
    for c in range(NC):
        m = dict(full)
        m["x"] = np.ascontiguousarray(full["x"][c * TPC:(c + 1) * TPC])
        in_maps.append(m)
    return in_maps


def kernel(**inputs) -> np.ndarray:
    from concourse import bass_utils
    nc = _get_program()
    in_maps = make_in_maps(inputs)
    res = bass_utils.run_bass_kernel_spmd(nc, in_maps, core_ids=list(range(NC)))
    out = np.concatenate([res.results[c]["out"] for c in range(NC)], axis=0)
    return out.reshape(B, S, D)


# revision 19
# speedup vs baseline: 2.3739x; 1.0440x over previous
"""Trainium2 Bass kernel for a GPT-2 style transformer block (v2, bf16).

Sharding (8 NeuronCores, SPMD-uniform program):
  - Tokens (B*S = 4096) sharded contiguously: core c owns tokens [512c, 512c+512).
  - Attention is head-sharded: core c computes heads {2c, 2c+1} over ALL tokens.
    AllToAlls exchange (Q^T, K^T) and V token-shards -> head-shards (split in two
    collectives so V compute overlaps the QK exchange), and O^T back.
  - All matmul operands are bf16 (fp32 PSUM accumulate); the residual stream
    stays fp32 in SBUF. LN scale/bias folded into following weights on host.
  - Weights live in SBUF slabs rotating through one shared pool tag so DMA
    prefetch of later phases overlaps earlier compute.
  - Attention K/V are SBUF-resident; scores are built transposed S^T[k, q], the
    exp output A^T feeds AV directly; lhsT = [ones | V_h] also produces softmax
    sums; normalization uses a PE broadcast matmul (no DRAM round-trip), with
    causal trimming of score/exp/AV free dims.
  - MLP runs in 8 fused blocks: fc (weight-stationary) -> gelu (one scalar
    activation instr, tanh approx) -> proj (h-stationary) accumulated into the
    fp32 residual in SBUF.
"""

import numpy as np

# ---------------------------------------------------------------- config

B, S, D, H = 2, 2048, 1024, 16
HD = D // H           # 64
FF = 4 * D            # 4096
NC = 8                # cores
TPC = B * S // NC     # 512 tokens per core
EPS = 1e-05

P = 128               # partitions
TT = TPC // P         # 4 token tiles per core
DK = D // P           # 8 contraction tiles over D
FFK = FF // P         # 32 tiles over FF
HPC = H // NC         # 2 heads per core
QB = TPC              # q-block width for attention (= shard width)
NQB = S // QB         # 4 q-blocks per batch
KPB = QB // P         # 4 k-tiles per q-block
NBLK = 8              # fused fc/proj blocks (512 ff features each)


def build_program():
    import contextlib

    import concourse.bass as bass
    import concourse.mybir as mybir
    import concourse.tile as tile
    from concourse import bacc
    from concourse.masks import make_identity, make_upper_triangular

    f32 = mybir.dt.float32
    f32r = mybir.dt.float32r
    bf16 = mybir.dt.bfloat16
    AF = mybir.ActivationFunctionType

    nc = bacc.Bacc("TRN2", target_bir_lowering=False, debug=False,
                   num_devices=NC)

    # ---- kernel I/O (per core) ----
    x_d = nc.dram_tensor("x", [TPC, D], f32, kind="ExternalInput").ap()
    caw_d = nc.dram_tensor("c_attn_w", [D, 3 * D], bf16, kind="ExternalInput").ap()
    cab_d = nc.dram_tensor("c_attn_b", [3 * D], f32, kind="ExternalInput").ap()
    cpw_d = nc.dram_tensor("c_proj_w", [D, D], bf16, kind="ExternalInput").ap()
    cpb_d = nc.dram_tensor("c_proj_b", [D], f32, kind="ExternalInput").ap()
    fcw_d = nc.dram_tensor("fc_w", [D, FF], bf16, kind="ExternalInput").ap()
    fcb_d = nc.dram_tensor("fc_b", [FF], f32, kind="ExternalInput").ap()
    pjw_d = nc.dram_tensor("proj_w", [FF, D], bf16, kind="ExternalInput").ap()
    pjb_d = nc.dram_tensor("proj_b", [D], f32, kind="ExternalInput").ap()
    out_d = nc.dram_tensor("out", [TPC, D], f32, kind="ExternalOutput").ap()

    PT = P * TPC  # elements in one [128, 512] slot region

    with tile.TileContext(nc) as tc:
        ctx = contextlib.ExitStack()
        with ctx:
            dram = ctx.enter_context(tc.tile_pool(name="dram", bufs=1,
                                                  space="DRAM"))
            consts = ctx.enter_context(tc.tile_pool(name="consts", bufs=1))
            wsl = ctx.enter_context(tc.tile_pool(name="wsl", bufs=13))
            resid = ctx.enter_context(tc.tile_pool(name="resid", bufs=1))
            acts = ctx.enter_context(tc.tile_pool(name="acts", bufs=1))
            attp = ctx.enter_context(tc.tile_pool(name="attp", bufs=1))
            temps = ctx.enter_context(tc.tile_pool(name="temps", bufs=4))
            stats = ctx.enter_context(tc.tile_pool(name="stats", bufs=2))
            psum = ctx.enter_context(tc.tile_pool(name="psum", bufs=1,
                                                  space="PSUM"))

            # a2a buffers (bf16)
            finQK = dram.tile([NC, 2 * PT], bf16)
            foutQK = dram.tile([NC, 2 * PT], bf16)
            finV = dram.tile([NC, PT], bf16)
            foutV = dram.tile([NC, PT], bf16)
            a2a_bin = dram.tile([NC, PT], bf16)
            a2a_bout = dram.tile([NC, PT], bf16)

            # ---------------- x first: it heads the critical path ----------
            x_tiles = [resid.tile([P, D], f32, tag=f"x{t}", name=f"x{t}")
                       for t in range(TT)]
            xr = x_d.rearrange("(t p) d -> t p d", p=P)
            for t in range(TT):
                nc.sync.dma_start(x_tiles[t], xr[t])

            # ---------------- constants ----------------
            ident = consts.tile([P, P], bf16)
            make_identity(nc, ident)
            # mask[k, q] = 1 if q >= k (within a diagonal 128x128 strip)
            mask_f = consts.tile([P, P], f32)
            make_upper_triangular(nc, mask_f, val=1.0, diag=True)
            mask2 = consts.tile([P, HPC, P], bf16)
            for h in range(HPC):
                nc.vector.tensor_copy(out=mask2[:, h, :], in_=mask_f)

            cab_qk = consts.tile([P, 2 * DK], f32)   # c_attn_b[0:2D] as [P, 16]
            nc.sync.dma_start(cab_qk, cab_d[0:2 * D].rearrange("(m p) -> p m", p=P))
            fcb_pp = consts.tile([P, FFK], f32)      # fc_b as [P, 32]
            nc.sync.dma_start(fcb_pp, fcb_d.rearrange("(m p) -> p m", p=P))

            def bcast_row(src_ap, off, n, name):
                t = consts.tile([P, n], f32, name=name)
                nc.sync.dma_start(t, bass.AP(
                    tensor=src_ap.tensor, offset=src_ap.offset + off,
                    ap=[[0, P], [1, n]]))
                return t

            vb_bc = bcast_row(cab_d, 2 * D, D, "vb_bc")    # V bias along feats
            cpb_bc = bcast_row(cpb_d, 0, D, "cpb_bc")
            pjb_bc = bcast_row(pjb_d, 0, D, "pjb_bc")

            # ---------------- weight slabs (shared rotating tag) ----------------
            # caw: 8 slabs [128, 3072]; cpw: 2 slabs [128, 4, 1024];
            # fcw: 8 m-slabs [128, 8, 512]; pjw: 8 slabs [128, 4, 1024].
            caw_r = caw_d.rearrange("(a p) f -> p a f", p=P)   # [128, 8, 3072]
            cpw_r = cpw_d.rearrange("(a p) d -> p a d", p=P)   # [128, 8, 1024]
            fcw_r = fcw_d.rearrange("(a p) f -> p a f", p=P)   # [128, 8, 4096]
            pjw_r = pjw_d.rearrange("(a p) d -> p a d", p=P)   # [128, 32, 1024]

            caw_sb = []
            for k in range(DK):
                t = wsl.tile([P, 3 * D], bf16, tag="wsl", name=f"caw{k}")
                nc.gpsimd.dma_start(t, caw_r[:, k, :])
                caw_sb.append(t)
            # cpw/fcw/pjw slab DMAs are emitted AFTER the forward collectives
            # on their queues so the collectives are not stuck behind their
            # slot-waits / descriptor generation.

            # ---------------- phase 1: LN1 + transpose ----------------
            def ln_transpose(dstT):
                """LayerNorm (ddof=1, eps on std) each [P, D] token tile of the
                residual, then PE-transpose into dstT [P, DK, TPC] (bf16)."""
                for t in range(TT):
                    xt = x_tiles[t]
                    st = stats.tile([P, 2, nc.vector.BN_STATS_DIM], f32,
                                    tag="bnst")
                    xg = xt.rearrange("p (g d) -> p g d", g=2)
                    for g in range(2):
                        nc.vector.bn_stats(out=st[:, g, :], in_=xg[:, g, :])
                    mv = stats.tile([P, nc.vector.BN_AGGR_DIM], f32, tag="mv")
                    nc.vector.bn_aggr(out=mv, in_=st)
                    sdev = stats.tile([P, 1], f32, tag="sdev")
                    nc.scalar.activation(out=sdev, in_=mv[:, 1:2], func=AF.Sqrt,
                                         scale=float(D) / (D - 1))
                    nc.vector.tensor_scalar_add(sdev, sdev, EPS)
                    rstd = stats.tile([P, 1], f32, tag="rstd")
                    nc.vector.reciprocal(out=rstd, in_=sdev)
                    nmr = stats.tile([P, 1], f32, tag="nmr")
                    nc.vector.tensor_scalar(out=nmr, in0=mv[:, 0:1],
                                            scalar1=rstd, scalar2=-1.0,
                                            op0=mybir.AluOpType.mult,
                                            op1=mybir.AluOpType.mult)
                    xn = temps.tile([P, D], bf16, tag="xn", bufs=2)
                    nc.scalar.activation(out=xn, in_=xt, func=AF.Identity,
                                         bias=nmr, scale=rstd)
                    for g in range(2):  # 4 transposes batched per eviction
                        pt = psum.tile([P, 4 * P], bf16, tag="ps", bufs=4,
                                       name="tp")
                        for i in range(4):
                            d = 4 * g + i
                            nc.tensor.transpose(pt[:, i * P:(i + 1) * P],
                                                xn[:, d * P:(d + 1) * P],
                                                ident)
                        nc.vector.tensor_copy(
                            out=dstT[:, 4 * g:4 * g + 4, t * P:(t + 1) * P],
                            in_=pt.rearrange("p (i c) -> p i c", c=P))

            xnT = acts.tile([P, DK, TPC], bf16, tag="xnT", name="xnT")
            ln_transpose(xnT)

            # ---------------- phase 2a: Q,K projections -> finQK ----------------
            for m in range(2 * DK):  # 16 feature tiles: Q then K
                ps = psum.tile([P, TPC], f32, tag="ps", bufs=4, name="mmps")
                for k in range(DK):
                    nc.tensor.matmul(ps, caw_sb[k][:, m * P:(m + 1) * P],
                                     xnT[:, k, :],
                                     start=(k == 0), stop=(k == DK - 1))
                sb = temps.tile([P, TPC], bf16, tag="ev", bufs=3, name="ev")
                nc.scalar.activation(out=sb, in_=ps, func=AF.Identity,
                                     bias=cab_qk[:, m:m + 1])
                j, half = m % DK, m // DK
                dst = finQK[j, half * PT:(half + 1) * PT].rearrange(
                    "(a b) -> a b", b=TPC)
                nc.sync.dma_start(dst, sb)

            # ---------------- phase 3a: forward AllToAll (Q,K) ----------------
            nc.gpsimd.collective_compute(
                "AllToAll", mybir.AluOpType.bypass,
                replica_groups=[list(range(NC))],
                ins=[finQK.opt()], outs=[foutQK.opt()])

            # ---------------- phase 2b: V projection -> finV ----------------
            for t in range(TT):
                for nb in range(2):
                    ns = D // 2
                    ps = psum.tile([P, ns], f32, tag="ps", bufs=4, name="mmps")
                    for k in range(DK):
                        nc.tensor.matmul(
                            ps, xnT[:, k, t * P:(t + 1) * P],
                            caw_sb[k][:, 2 * D + nb * ns:2 * D + (nb + 1) * ns],
                            start=(k == 0), stop=(k == DK - 1))
                    vt = temps.tile([P, ns], bf16, tag="vt", bufs=2, name="vt")
                    nc.vector.tensor_add(out=vt, in0=ps,
                                         in1=vb_bc[:, nb * ns:(nb + 1) * ns])
                    # one DMA covering the 4 destination slots
                    nc.sync.dma_start(
                        finV[nb * 4:(nb + 1) * 4,
                             t * P * P:(t + 1) * P * P].rearrange(
                            "j (p f) -> p j f", p=P),
                        vt.rearrange("p (j f) -> p j f", f=P))

            # ---------------- phase 3b: forward AllToAll (V) ----------------
            nc.gpsimd.collective_compute(
                "AllToAll", mybir.AluOpType.bypass,
                replica_groups=[list(range(NC))],
                ins=[finV.opt()], outs=[foutV.opt()])

            # cpw/fcw slabs: issued on gpsimd after the forward collectives
            # (slots are free once QKV released the caw slabs; the bwd
            # collective behind them is not needed until attention ends).
            cpw_sb = []
            for c in range(2):
                t = wsl.tile([P, 4, D], bf16, tag="wsl", name=f"cpw{c}")
                nc.gpsimd.dma_start(t, cpw_r[:, 4 * c:4 * c + 4, :])
                cpw_sb.append(t)
            fcw_sb = []
            for g in range(NBLK):
                t = wsl.tile([P, DK, 512], bf16, tag="wsl", name=f"fcw{g}")
                nc.gpsimd.dma_start(t, fcw_r[:, :, g * 512:(g + 1) * 512])
                fcw_sb.append(t)

            # ---------------- phase 4: attention (my 2 heads, all tokens) ----
            # K^T resident: [128 feat, slot, 512 tok]
            ktr = attp.tile([P, NC, TPC], bf16, tag="ktr", name="ktr")
            nc.sync.dma_start(
                ktr, foutQK.rearrange("n (r p t) -> r p n t", r=2, p=P)[1])
            # V resident with ones col per head: [128 tok, slot, t, 2*(64+1)]
            # foutV lands contiguously in vtmp (big DMA descriptors); a vector
            # repack inserts the per-head layout (tiny DMA descriptors would
            # cost ~25us otherwise).
            vres = attp.tile([P, NC, TT, HPC * (HD + 1)], bf16, tag="vres",
                             name="vres")
            for j in range(NC):
                nc.vector.memset(
                    vres[:, j, :, :].rearrange("p t (h c) -> p t h c",
                                               c=HD + 1)[:, :, :, HD:HD + 1],
                    1.0)
                vtmp = attp.tile([P, TT, P], bf16, tag="vtmp", bufs=2,
                                 name="vtmp")
                nc.sync.dma_start(
                    vtmp, foutV[j, :].rearrange("(t p f) -> p t f", t=TT, p=P))
                for t in range(TT):
                    nc.vector.tensor_copy(
                        out=vres[:, j, t, :].rearrange(
                            "p (h c) -> p h c", c=HD + 1)[:, :, 0:HD],
                        in_=vtmp[:, t, :].rearrange("p (h c) -> p h c", c=HD))

            isc = 1.0 / float(np.sqrt(HD))
            for b in range(B):
                for qb in range(NQB):
                    slot_q = b * NQB + qb
                    qt = attp.tile([P, QB], bf16, tag="qt", bufs=2, name="qt")
                    nc.sync.dma_start(
                        qt, foutQK[slot_q, 0:PT].rearrange("(a b) -> a b",
                                                           b=TPC))
                    op = psum.tile([HD + 1, HPC * QB], f32, tag="ps", bufs=4,
                                   name="op")
                    nkt = (qb + 1) * KPB
                    for kt in range(nkt):
                        src_slot = b * NQB + kt // KPB
                        off = (kt % KPB) * P
                        d = kt - qb * KPB  # >= 0 on diagonal strips
                        qs = max(d, 0) * P
                        sps = psum.tile([P, HPC * QB], f32, tag="ps", bufs=4,
                                        name="sps")
                        for h in range(HPC):
                            nc.tensor.matmul(
                                sps[:, h * QB + qs:(h + 1) * QB],
                                ktr[h * HD:(h + 1) * HD, src_slot,
                                    off:off + P],
                                qt[h * HD:(h + 1) * HD, qs:QB],
                                start=True, stop=True)
                        at = attp.tile([P, HPC, QB], bf16, tag="at", bufs=3,
                                       name="at")
                        spsv = sps.rearrange("p (h q) -> p h q", h=HPC)
                        nc.scalar.activation(out=at[:, :, qs:QB],
                                             in_=spsv[:, :, qs:QB],
                                             func=AF.Exp, scale=isc)
                        if d >= 0:
                            nc.vector.tensor_mul(out=at[:, :, qs:qs + P],
                                                 in0=at[:, :, qs:qs + P],
                                                 in1=mask2)
                        for h in range(HPC):
                            nc.tensor.matmul(
                                op[:, h * QB + qs:(h + 1) * QB],
                                vres[:, src_slot, kt % KPB,
                                     h * (HD + 1):(h + 1) * (HD + 1)],
                                at[:, h, qs:QB],
                                start=(kt == 0), stop=(kt == nkt - 1))
                    # normalize: recip of sums row, partition-broadcast via a
                    # DRAM bounce (hidden by op double-buffering)
                    rc = attp.tile([1, HPC * QB], f32, tag="rc", bufs=2,
                                   name="rc")
                    nc.vector.reciprocal(out=rc, in_=op[HD:HD + 1, :])
                    rcd = dram.tile([HPC * QB], f32, tag="rcd", name="rcd",
                                    bufs=2)
                    nc.sync.dma_start(rcd, rc)
                    rbc = attp.tile([HD, HPC * QB], f32, tag="rbc", bufs=2,
                                    name="rbc")
                    nc.sync.dma_start(rbc, bass.AP(
                        tensor=rcd.tensor, offset=rcd.offset,
                        ap=[[0, HD], [1, HPC * QB]]))
                    otv = attp.tile([HD, HPC * QB], bf16, tag="otv", bufs=2,
                                    name="otv")
                    nc.vector.tensor_mul(out=otv, in0=op[0:HD, :], in1=rbc)
                    for h in range(HPC):
                        nc.sync.dma_start(
                            a2a_bin[slot_q, h * HD * TPC:(h + 1) * HD * TPC]
                            .rearrange("(a b) -> a b", b=TPC),
                            otv[:, h * QB:(h + 1) * QB])

            # ---------------- phase 3c: backward AllToAll (O^T) ----------------
            nc.gpsimd.collective_compute(
                "AllToAll", mybir.AluOpType.bypass,
                replica_groups=[list(range(NC))],
                ins=[a2a_bin.opt()], outs=[a2a_bout.opt()])

            # ---------------- phase 5: attn out-proj + residual (in place) ----
            otf = acts.tile([P, DK, TPC], bf16, tag="xnT", name="otf")
            nc.sync.dma_start(otf, a2a_bout[:].rearrange(
                "n (p t) -> p n t", p=P))
            # pjw slabs on the sync queue here: the only sync work emitted
            # after them is the final out DMA, so their slot-waits (on caw/cpw/
            # fcw slab releases, all PE-driven) cannot head-of-line block any
            # compute. Slabs 0-4 bind to slots already free by this point;
            # 5-7 bind as the first fc blocks release their fcw slabs.
            pjw_sb = []
            for g in range(NBLK):
                t = wsl.tile([P, 4, D], bf16, tag="wsl", name=f"pjw{g}")
                nc.sync.dma_start(t, pjw_r[:, 4 * g:4 * g + 4, :])
                pjw_sb.append(t)
            for t in range(TT):
                nc.vector.tensor_add(out=x_tiles[t], in0=x_tiles[t],
                                     in1=cpb_bc)
            for t in range(TT):
                for nb in range(2):
                    ns = D // 2
                    ps = psum.tile([P, ns], f32, tag="ps", bufs=4, name="mmps")
                    for k in range(DK):
                        nc.tensor.matmul(
                            ps, otf[:, k, t * P:(t + 1) * P],
                            cpw_sb[k // 4][:, k % 4, nb * ns:(nb + 1) * ns],
                            start=(k == 0), stop=(k == DK - 1))
                    sl = slice(nb * ns, (nb + 1) * ns)
                    nc.vector.tensor_add(out=x_tiles[t][:, sl], in0=ps,
                                         in1=x_tiles[t][:, sl])

            # ---------------- phase 6: LN2 + transpose ----------------
            xn2T = acts.tile([P, DK, TPC], bf16, tag="xnT", name="xn2T")
            ln_transpose(xn2T)

            # ---------------- phase 7+8: fused MLP blocks ----------------
            for t in range(TT):
                nc.vector.tensor_add(out=x_tiles[t], in0=x_tiles[t],
                                     in1=pjb_bc)
            for g in range(NBLK):
                ht = []
                for mm in range(4):
                    m = g * 4 + mm
                    ps = psum.tile([P, TPC], f32, tag="ps", bufs=4,
                                   name="mmps")
                    for k in range(DK):
                        nc.tensor.matmul(
                            ps, fcw_sb[g][:, k, mm * P:(mm + 1) * P],
                            xn2T[:, k, :],
                            start=(k == 0), stop=(k == DK - 1))
                    hm = temps.tile([P, TPC], bf16, tag="hT", bufs=8,
                                    name="hT")
                    nc.scalar.activation(out=hm, in_=ps,
                                         func=AF.Gelu_apprx_tanh,
                                         bias=fcb_pp[:, m:m + 1])
                    ht.append(hm)
                for t in range(TT):
                    for nb in range(2):
                        ns = D // 2
                        ps = psum.tile([P, ns], f32, tag="ps", bufs=4,
                                       name="mmps")
                        for kk in range(4):
                            nc.tensor.matmul(
                                ps, ht[kk][:, t * P:(t + 1) * P],
                                pjw_sb[g][:, kk, nb * ns:(nb + 1) * ns],
                                start=(kk == 0), stop=(kk == 3))
                        sl = slice(nb * ns, (nb + 1) * ns)
                        nc.vector.tensor_add(out=x_tiles[t][:, sl], in0=ps,
                                             in1=x_tiles[t][:, sl])

            # ---------------- output ----------------
            outr = out_d.rearrange("(t p) d -> t p d", p=P)
            for t in range(TT):
                nc.sync.dma_start(outr[t], x_tiles[t])

    nc.compile()
    return nc


_NC_CACHE = None


def _get_program():
    global _NC_CACHE
    if _NC_CACHE is None:
        _NC_CACHE = build_program()
    return _NC_CACHE


def host_fold(inputs):
    """Fold LN scale/bias into the following matmul weights; cast weights to
    bf16 (host side)."""
    import ml_dtypes
    bf = ml_dtypes.bfloat16

    def f(a):
        return np.ascontiguousarray(np.asarray(a), dtype=np.float32)
    x = f(inputs["x"]).reshape(B * S, D)
    caw0 = f(inputs["c_attn_w"])
    fcw0 = f(inputs["fc_w"])
    caw = caw0 * f(inputs["ln1_w"])[:, None]
    cab = f(inputs["c_attn_b"]) + f(inputs["ln1_b"]) @ caw0
    fcw = fcw0 * f(inputs["ln2_w"])[:, None]
    fcb = f(inputs["fc_b"]) + f(inputs["ln2_b"]) @ fcw0

    def w(a):
        return np.ascontiguousarray(np.asarray(a, dtype=bf))
    return {
        "x": x,
        "c_attn_w": w(caw), "c_attn_b": f(cab),
        "c_proj_w": w(inputs["c_proj_w"]), "c_proj_b": f(inputs["c_proj_b"]),
        "fc_w": w(fcw), "fc_b": f(fcb),
        "proj_w": w(inputs["proj_w"]), "proj_b": f(inputs["proj_b"]),
    }


def make_in_maps(inputs):
    full = host_fold(inputs)
    in_maps = []
    for c in range(NC):
        m = dict(full)
        m["x"] = np.ascontiguousarray(full["x"][c * TPC:(c + 1) * TPC])
        in_maps.append(m)
    return in_maps


def kernel(**inputs) -> np.ndarray:
    from concourse import bass_utils
    nc = _get_program()
    in_maps = make_in_maps(inputs)
    res = bass_utils.run_bass_kernel_spmd(nc, in_maps, core_ids=list(range(NC)))
    out = np.concatenate([res.results[c]["out"] for c in range(NC)], axis=0)
    return out.reshape(B, S, D)


# revision 28
# speedup vs baseline: 2.3864x; 1.0053x over previous
"""Trainium2 Bass kernel for a GPT-2 style transformer block (v2, bf16).

Sharding (8 NeuronCores, SPMD-uniform program):
  - Tokens (B*S = 4096) sharded contiguously: core c owns tokens [512c, 512c+512).
  - Attention is head-sharded: core c computes heads {2c, 2c+1} over ALL tokens.
    AllToAlls exchange (Q^T, K^T) and V token-shards -> head-shards (split in two
    collectives so V compute overlaps the QK exchange), and O^T back.
  - All matmul operands are bf16 (fp32 PSUM accumulate); the residual stream
    stays fp32 in SBUF. LN scale/bias folded into following weights on host.
  - Weights live in SBUF slabs rotating through one shared pool tag so DMA
    prefetch of later phases overlaps earlier compute.
  - Attention K/V are SBUF-resident; scores are built transposed S^T[k, q], the
    exp output A^T feeds AV directly; lhsT = [ones | V_h] also produces softmax
    sums; normalization uses a PE broadcast matmul (no DRAM round-trip), with
    causal trimming of score/exp/AV free dims.
  - MLP runs in 8 fused blocks: fc (weight-stationary) -> gelu (one scalar
    activation instr, tanh approx) -> proj (h-stationary) accumulated into the
    fp32 residual in SBUF.
"""

import numpy as np

# ---------------------------------------------------------------- config

B, S, D, H = 2, 2048, 1024, 16
HD = D // H           # 64
FF = 4 * D            # 4096
NC = 8                # cores
TPC = B * S // NC     # 512 tokens per core
EPS = 1e-05

P = 128               # partitions
TT = TPC // P         # 4 token tiles per core
DK = D // P           # 8 contraction tiles over D
FFK = FF // P         # 32 tiles over FF
HPC = H // NC         # 2 heads per core
QB = TPC              # q-block width for attention (= shard width)
NQB = S // QB         # 4 q-blocks per batch
KPB = QB // P         # 4 k-tiles per q-block
NBLK = 8              # fused fc/proj blocks (512 ff features each)


def build_program():
    import contextlib

    import concourse.bass as bass
    import concourse.mybir as mybir
    import concourse.tile as tile
    from concourse import bacc
    from concourse.masks import make_identity, make_upper_triangular

    f32 = mybir.dt.float32
    f32r = mybir.dt.float32r
    bf16 = mybir.dt.bfloat16
    AF = mybir.ActivationFunctionType

    nc = bacc.Bacc("TRN2", target_bir_lowering=False, debug=False,
                   num_devices=NC)

    # ---- kernel I/O (per core) ----
    x_d = nc.dram_tensor("x", [TPC, D], f32, kind="ExternalInput").ap()
    caw_d = nc.dram_tensor("c_attn_w", [D, 3 * D], bf16, kind="ExternalInput").ap()
    cab_d = nc.dram_tensor("c_attn_b", [3 * D], f32, kind="ExternalInput").ap()
    cpw_d = nc.dram_tensor("c_proj_w", [D, D], bf16, kind="ExternalInput").ap()
    cpb_d = nc.dram_tensor("c_proj_b", [D], f32, kind="ExternalInput").ap()
    fcw_d = nc.dram_tensor("fc_w", [D, FF], bf16, kind="ExternalInput").ap()
    fcb_d = nc.dram_tensor("fc_b", [FF], f32, kind="ExternalInput").ap()
    pjw_d = nc.dram_tensor("proj_w", [FF, D], bf16, kind="ExternalInput").ap()
    pjb_d = nc.dram_tensor("proj_b", [D], f32, kind="ExternalInput").ap()
    out_d = nc.dram_tensor("out", [TPC, D], f32, kind="ExternalOutput").ap()

    PT = P * TPC  # elements in one [128, 512] slot region

    with tile.TileContext(nc) as tc:
        ctx = contextlib.ExitStack()
        with ctx:
            dram = ctx.enter_context(tc.tile_pool(name="dram", bufs=1,
                                                  space="DRAM"))
            consts = ctx.enter_context(tc.tile_pool(name="consts", bufs=1))
            wsl = ctx.enter_context(tc.tile_pool(name="wsl", bufs=13))
            resid = ctx.enter_context(tc.tile_pool(name="resid", bufs=1))
            acts = ctx.enter_context(tc.tile_pool(name="acts", bufs=1))
            attp = ctx.enter_context(tc.tile_pool(name="attp", bufs=1))
            temps = ctx.enter_context(tc.tile_pool(name="temps", bufs=4))
            stats = ctx.enter_context(tc.tile_pool(name="stats", bufs=2))
            psum = ctx.enter_context(tc.tile_pool(name="psum", bufs=1,
                                                  space="PSUM"))

            # a2a buffers (bf16)
            finQK = dram.tile([NC, 2 * PT], bf16)
            foutQK = dram.tile([NC, 2 * PT], bf16)
            finV = dram.tile([NC, PT], bf16)
            foutV = dram.tile([NC, PT], bf16)
            a2a_bin = dram.tile([NC, PT], bf16)
            a2a_bout = dram.tile([NC, PT], bf16)

            # ---------------- x first: it heads the critical path ----------
            x_tiles = [resid.tile([P, D], f32, tag=f"x{t}", name=f"x{t}")
                       for t in range(TT)]
            xr = x_d.rearrange("(t p) d -> t p d", p=P)
            for t in range(TT):
                nc.sync.dma_start(x_tiles[t], xr[t])

            # ---------------- constants ----------------
            ident = consts.tile([P, P], bf16)
            make_identity(nc, ident)
            # mask[k, q] = 1 if q >= k (within a diagonal 128x128 strip)
            mask_f = consts.tile([P, P], f32)
            make_upper_triangular(nc, mask_f, val=1.0, diag=True)
            mask2 = consts.tile([P, HPC, P], bf16)
            for h in range(HPC):
                nc.vector.tensor_copy(out=mask2[:, h, :], in_=mask_f)

            ones_f = consts.tile([1, HD], f32)
            nc.vector.memset(ones_f, 1.0)
            ones64 = consts.tile([1, HD], f32r)
            nc.vector.tensor_copy(out=ones64, in_=ones_f)

            # per-partition bias tile, pre-transposed on host (a strided DMA
            # here would emit thousands of 4-byte descriptors)
            bias_d = nc.dram_tensor("bias_pp", [P, 2 * DK + FFK], f32,
                                    kind="ExternalInput").ap()
            bias_pp = consts.tile([P, 2 * DK + FFK], f32)
            nc.sync.dma_start(bias_pp, bias_d)
            cab_qk = bias_pp[:, 0:2 * DK]
            fcb_pp = bias_pp[:, 2 * DK:]

            def bcast_row(src_ap, off, n, name):
                t = consts.tile([P, n], f32, name=name)
                nc.sync.dma_start(t, bass.AP(
                    tensor=src_ap.tensor, offset=src_ap.offset + off,
                    ap=[[0, P], [1, n]]))
                return t

            # ---------------- weight slabs (shared rotating tag) ----------------
            # caw: 8 slabs [128, 3072]; cpw: 2 slabs [128, 4, 1024];
            # fcw: 8 m-slabs [128, 8, 512]; pjw: 8 slabs [128, 4, 1024].
            caw_r = caw_d.rearrange("(a p) f -> p a f", p=P)   # [128, 8, 3072]
            cpw_r = cpw_d.rearrange("(a p) d -> p a d", p=P)   # [128, 8, 1024]
            fcw_r = fcw_d.rearrange("(a p) f -> p a f", p=P)   # [128, 8, 4096]
            pjw_r = pjw_d.rearrange("(a p) d -> p a d", p=P)   # [128, 32, 1024]

            caw_sb = []
            for k in range(DK):
                t = wsl.tile([P, 3 * D], bf16, tag="wsl", name=f"caw{k}")
                nc.gpsimd.dma_start(t, caw_r[:, k, :])
                caw_sb.append(t)
            # cpw/fcw/pjw slab DMAs are emitted AFTER the forward collectives
            # on their queues so the collectives are not stuck behind their
            # slot-waits / descriptor generation.

            # ---------------- phase 1: LN1 + transpose ----------------
            def ln_transpose(dstT):
                """LayerNorm (ddof=1, eps on std) each [P, D] token tile of the
                residual, then PE-transpose into dstT [P, DK, TPC] (bf16)."""
                for t in range(TT):
                    xt = x_tiles[t]
                    st = stats.tile([P, 2, nc.vector.BN_STATS_DIM], f32,
                                    tag="bnst")
                    xg = xt.rearrange("p (g d) -> p g d", g=2)
                    for g in range(2):
                        nc.vector.bn_stats(out=st[:, g, :], in_=xg[:, g, :])
                    mv = stats.tile([P, nc.vector.BN_AGGR_DIM], f32, tag="mv")
                    nc.vector.bn_aggr(out=mv, in_=st)
                    sdev = stats.tile([P, 1], f32, tag="sdev")
                    nc.scalar.activation(out=sdev, in_=mv[:, 1:2], func=AF.Sqrt,
                                         scale=float(D) / (D - 1))
                    nc.vector.tensor_scalar_add(sdev, sdev, EPS)
                    rstd = stats.tile([P, 1], f32, tag="rstd")
                    nc.vector.reciprocal(out=rstd, in_=sdev)
                    nmr = stats.tile([P, 1], f32, tag="nmr")
                    nc.vector.tensor_scalar(out=nmr, in0=mv[:, 0:1],
                                            scalar1=rstd, scalar2=-1.0,
                                            op0=mybir.AluOpType.mult,
                                            op1=mybir.AluOpType.mult)
                    xn = temps.tile([P, D], bf16, tag="xn", bufs=2)
                    nc.scalar.activation(out=xn, in_=xt, func=AF.Identity,
                                         bias=nmr, scale=rstd)
                    for g in range(2):  # 4 transposes batched per eviction
                        pt = psum.tile([P, 4 * P], bf16, tag="ps", bufs=4,
                                       name="tp")
                        for i in range(4):
                            d = 4 * g + i
                            nc.tensor.transpose(pt[:, i * P:(i + 1) * P],
                                                xn[:, d * P:(d + 1) * P],
                                                ident)
                        nc.vector.tensor_copy(
                            out=dstT[:, 4 * g:4 * g + 4, t * P:(t + 1) * P],
                            in_=pt.rearrange("p (i c) -> p i c", c=P))

            xnT = acts.tile([P, DK, TPC], bf16, tag="xnT", name="xnT")
            ln_transpose(xnT)

            # ---------------- phase 2a: Q,K projections -> finQK ----------------
            for m in range(2 * DK):  # 16 feature tiles: Q then K
                ps = psum.tile([P, TPC], f32, tag="ps", bufs=4, name="mmps")
                for k in range(DK):
                    nc.tensor.matmul(ps, caw_sb[k][:, m * P:(m + 1) * P],
                                     xnT[:, k, :],
                                     start=(k == 0), stop=(k == DK - 1))
                sb = temps.tile([P, TPC], bf16, tag="ev", bufs=3, name="ev")
                nc.scalar.activation(out=sb, in_=ps, func=AF.Identity,
                                     bias=cab_qk[:, m:m + 1])
                j, half = m % DK, m // DK
                dst = finQK[j, half * PT:(half + 1) * PT].rearrange(
                    "(a b) -> a b", b=TPC)
                nc.sync.dma_start(dst, sb)

            # ---------------- phase 3a: forward AllToAll (Q,K) ----------------
            nc.gpsimd.collective_compute(
                "AllToAll", mybir.AluOpType.bypass,
                replica_groups=[list(range(NC))],
                ins=[finQK.opt()], outs=[foutQK.opt()])

            # ---------------- phase 2b: V projection -> finV ----------------
            vb_bc = bcast_row(cab_d, 2 * D, D, "vb_bc")    # V bias along feats
            for t in range(TT):
                ps = psum.tile([P, D], f32, tag="ps", bufs=4, name="mmps")
                for nb in range(2):
                    ns = D // 2
                    for k in range(DK):
                        nc.tensor.matmul(
                            ps[:, nb * ns:(nb + 1) * ns],
                            xnT[:, k, t * P:(t + 1) * P],
                            caw_sb[k][:, 2 * D + nb * ns:2 * D + (nb + 1) * ns],
                            start=(k == 0), stop=(k == DK - 1))
                vt = temps.tile([P, D], bf16, tag="vt", bufs=2, name="vt")
                nc.vector.tensor_add(out=vt, in0=ps, in1=vb_bc)
                # one DMA covering all 8 destination slots
                nc.sync.dma_start(
                    finV[:, t * P * P:(t + 1) * P * P].rearrange(
                        "j (p f) -> p j f", p=P),
                    vt.rearrange("p (j f) -> p j f", f=P))

            # ---------------- phase 3b: forward AllToAll (V) ----------------
            nc.gpsimd.collective_compute(
                "AllToAll", mybir.AluOpType.bypass,
                replica_groups=[list(range(NC))],
                ins=[finV.opt()], outs=[foutV.opt()])

            # cpw/fcw slabs: issued on gpsimd after the forward collectives
            # (slots are free once QKV released the caw slabs; the bwd
            # collective behind them is not needed until attention ends).
            cpw_sb = []
            for c in range(2):
                t = wsl.tile([P, 4, D], bf16, tag="wsl", name=f"cpw{c}")
                nc.gpsimd.dma_start(t, cpw_r[:, 4 * c:4 * c + 4, :])
                cpw_sb.append(t)
            fcw_sb = []
            for g in range(NBLK):
                t = wsl.tile([P, DK, 512], bf16, tag="wsl", name=f"fcw{g}")
                nc.gpsimd.dma_start(t, fcw_r[:, :, g * 512:(g + 1) * 512])
                fcw_sb.append(t)

            # ---------------- phase 4: attention (my 2 heads, all tokens) ----
            # K^T resident: [128 feat, slot, 512 tok]
            ktr = attp.tile([P, NC, TPC], bf16, tag="ktr", name="ktr")
            nc.sync.dma_start(
                ktr, foutQK.rearrange("n (r p t) -> r p n t", r=2, p=P)[1])
            # V resident with ones col per head: [128 tok, slot, t, 2*(64+1)]
            # foutV lands contiguously in vtmp (big DMA descriptors); a vector
            # repack inserts the per-head layout (tiny DMA descriptors would
            # cost ~25us otherwise).
            vres = attp.tile([P, NC, TT, HPC * (HD + 1)], bf16, tag="vres",
                             name="vres")
            for j in range(NC):
                nc.vector.memset(
                    vres[:, j, :, :].rearrange("p t (h c) -> p t h c",
                                               c=HD + 1)[:, :, :, HD:HD + 1],
                    1.0)
                vtmp = attp.tile([P, TT, P], bf16, tag="vtmp", bufs=2,
                                 name="vtmp")
                nc.sync.dma_start(
                    vtmp, foutV[j, :].rearrange("(t p f) -> p t f", t=TT, p=P))
                for t in range(TT):
                    nc.vector.tensor_copy(
                        out=vres[:, j, t, :].rearrange(
                            "p (h c) -> p h c", c=HD + 1)[:, :, 0:HD],
                        in_=vtmp[:, t, :].rearrange("p (h c) -> p h c", c=HD))

            isc = 1.0 / float(np.sqrt(HD))
            for b in range(B):
                for qb in range(NQB):
                    slot_q = b * NQB + qb
                    qt = attp.tile([P, QB], bf16, tag="qt", bufs=2, name="qt")
                    nc.sync.dma_start(
                        qt, foutQK[slot_q, 0:PT].rearrange("(a b) -> a b",
                                                           b=TPC))
                    op = psum.tile([HD + 1, HPC * QB], f32, tag="ps", bufs=4,
                                   name="op")
                    nkt = (qb + 1) * KPB
                    for kt in range(nkt):
                        src_slot = b * NQB + kt // KPB
                        off = (kt % KPB) * P
                        d = kt - qb * KPB  # >= 0 on diagonal strips
                        qs = max(d, 0) * P
                        sps = psum.tile([P, HPC * QB], f32, tag="ps", bufs=4,
                                        name="sps")
                        for h in range(HPC):
                            nc.tensor.matmul(
                                sps[:, h * QB + qs:(h + 1) * QB],
                                ktr[h * HD:(h + 1) * HD, src_slot,
                                    off:off + P],
                                qt[h * HD:(h + 1) * HD, qs:QB],
                                start=True, stop=True)
                        at = attp.tile([P, HPC, QB], bf16, tag="at", bufs=3,
                                       name="at")
                        spsv = sps.rearrange("p (h q) -> p h q", h=HPC)
                        nc.scalar.activation(out=at[:, :, qs:QB],
                                             in_=spsv[:, :, qs:QB],
                                             func=AF.Exp, scale=isc)
                        if d >= 0:
                            nc.vector.tensor_mul(out=at[:, :, qs:qs + P],
                                                 in0=at[:, :, qs:qs + P],
                                                 in1=mask2)
                        for h in range(HPC):
                            nc.tensor.matmul(
                                op[:, h * QB + qs:(h + 1) * QB],
                                vres[:, src_slot, kt % KPB,
                                     h * (HD + 1):(h + 1) * (HD + 1)],
                                at[:, h, qs:QB],
                                start=(kt == 0), stop=(kt == nkt - 1))
                    # normalize: recip of the sums row, partition-broadcast
                    # via a cheap PE matmul (ones64^T @ rc), O^T copied to
                    # SBUF so the multiply has a single PSUM operand
                    rc = attp.tile([1, HPC * QB], f32r, tag="rc", bufs=2,
                                   name="rc")
                    with nc.allow_low_precision(reason="f32r == f32 bitwise"):
                        nc.vector.reciprocal(out=rc, in_=op[HD:HD + 1, :])
                    otu = attp.tile([HD, HPC * QB], bf16, tag="otu", bufs=2,
                                    name="otu")
                    nc.vector.tensor_copy(out=otu, in_=op[0:HD, :])
                    otv = attp.tile([HD, HPC * QB], bf16, tag="otv", bufs=2,
                                    name="otv")
                    for h in range(HPC):
                        bc = psum.tile([HD, QB], f32, tag="ps", bufs=4,
                                       name="bc")
                        nc.tensor.matmul(bc, ones64,
                                         rc[:, h * QB:(h + 1) * QB],
                                         start=True, stop=True)
                        nc.vector.tensor_mul(
                            out=otv[:, h * QB:(h + 1) * QB],
                            in0=otu[:, h * QB:(h + 1) * QB], in1=bc)
                        nc.sync.dma_start(
                            a2a_bin[slot_q, h * HD * TPC:(h + 1) * HD * TPC]
                            .rearrange("(a b) -> a b", b=TPC),
                            otv[:, h * QB:(h + 1) * QB])

            # ---------------- phase 3c: backward AllToAll (O^T) ----------------
            nc.gpsimd.collective_compute(
                "AllToAll", mybir.AluOpType.bypass,
                replica_groups=[list(range(NC))],
                ins=[a2a_bin.opt()], outs=[a2a_bout.opt()])

            # ---------------- phase 5: attn out-proj + residual (in place) ----
            otf = acts.tile([P, DK, TPC], bf16, tag="xnT", name="otf")
            nc.sync.dma_start(otf, a2a_bout[:].rearrange(
                "n (p t) -> p n t", p=P))
            # pjw slabs on the sync queue here: the only sync work emitted
            # after them is the final out DMA, so their slot-waits (on caw/cpw/
            # fcw slab releases, all PE-driven) cannot head-of-line block any
            # compute. Slabs 0-4 bind to slots already free by this point;
            # 5-7 bind as the first fc blocks release their fcw slabs.
            pjw_sb = []
            for g in range(NBLK):
                t = wsl.tile([P, 4, D], bf16, tag="wsl", name=f"pjw{g}")
                nc.sync.dma_start(t, pjw_r[:, 4 * g:4 * g + 4, :])
                pjw_sb.append(t)
            cpb_bc = bcast_row(cpb_d, 0, D, "cpb_bc")
            pjb_bc = bcast_row(pjb_d, 0, D, "pjb_bc")
            for t in range(TT):
                nc.vector.tensor_add(out=x_tiles[t], in0=x_tiles[t],
                                     in1=cpb_bc)
            for t in range(TT):
                ps = psum.tile([P, D], f32, tag="ps", bufs=4, name="mmps")
                for nb in range(2):
                    ns = D // 2
                    for k in range(DK):
                        nc.tensor.matmul(
                            ps[:, nb * ns:(nb + 1) * ns],
                            otf[:, k, t * P:(t + 1) * P],
                            cpw_sb[k // 4][:, k % 4, nb * ns:(nb + 1) * ns],
                            start=(k == 0), stop=(k == DK - 1))
                nc.vector.tensor_add(out=x_tiles[t], in0=ps, in1=x_tiles[t])

            # ---------------- phase 6: LN2 + transpose ----------------
            xn2T = acts.tile([P, DK, TPC], bf16, tag="xnT", name="xn2T")
            ln_transpose(xn2T)

            # ---------------- phase 7+8: fused MLP blocks ----------------
            for t in range(TT):
                nc.vector.tensor_add(out=x_tiles[t], in0=x_tiles[t],
                                     in1=pjb_bc)
            for g in range(NBLK):
                ht = []
                for mm in range(4):
                    m = g * 4 + mm
                    ps = psum.tile([P, TPC], f32, tag="ps", bufs=4,
                                   name="mmps")
                    for k in range(DK):
                        nc.tensor.matmul(
                            ps, fcw_sb[g][:, k, mm * P:(mm + 1) * P],
                            xn2T[:, k, :],
                            start=(k == 0), stop=(k == DK - 1))
                    hm = temps.tile([P, TPC], bf16, tag="hT", bufs=8,
                                    name="hT")
                    nc.scalar.activation(out=hm, in_=ps,
                                         func=AF.Gelu_apprx_tanh,
                                         bias=fcb_pp[:, m:m + 1])
                    ht.append(hm)
                for t in range(TT):
                    ps = psum.tile([P, D], f32, tag="ps", bufs=4,
                                   name="mmps")
                    for nb in range(2):
                        ns = D // 2
                        for kk in range(4):
                            nc.tensor.matmul(
                                ps[:, nb * ns:(nb + 1) * ns],
                                ht[kk][:, t * P:(t + 1) * P],
                                pjw_sb[g][:, kk, nb * ns:(nb + 1) * ns],
                                start=(kk == 0), stop=(kk == 3))
                    nc.vector.tensor_add(out=x_tiles[t], in0=ps,
                                         in1=x_tiles[t])

            # ---------------- output ----------------
            outr = out_d.rearrange("(t p) d -> t p d", p=P)
            for t in range(TT):
                nc.sync.dma_start(outr[t], x_tiles[t])

    nc.compile()
    return nc


_NC_CACHE = None


def _get_program():
    global _NC_CACHE
    if _NC_CACHE is None:
        _NC_CACHE = build_program()
    return _NC_CACHE


def host_fold(inputs):
    """Fold LN scale/bias into the following matmul weights; cast weights to
    bf16 (host side)."""
    import ml_dtypes
    bf = ml_dtypes.bfloat16

    def f(a):
        return np.ascontiguousarray(np.asarray(a), dtype=np.float32)
    x = f(inputs["x"]).reshape(B * S, D)
    caw0 = f(inputs["c_attn_w"])
    fcw0 = f(inputs["fc_w"])
    caw = caw0 * f(inputs["ln1_w"])[:, None]
    cab = f(inputs["c_attn_b"]) + f(inputs["ln1_b"]) @ caw0
    fcw = fcw0 * f(inputs["ln2_w"])[:, None]
    fcb = f(inputs["fc_b"]) + f(inputs["ln2_b"]) @ fcw0

    def w(a):
        return np.ascontiguousarray(np.asarray(a, dtype=bf))
    # per-partition bias tile [128, 48]: QK biases then fc biases, transposed
    bias_pp = np.concatenate([cab[:2 * D].reshape(2 * DK, P).T,
                              fcb.reshape(FFK, P).T], axis=1)
    return {
        "x": x,
        "c_attn_w": w(caw), "c_attn_b": f(cab),
        "c_proj_w": w(inputs["c_proj_w"]), "c_proj_b": f(inputs["c_proj_b"]),
        "fc_w": w(fcw), "fc_b": f(fcb),
        "proj_w": w(inputs["proj_w"]), "proj_b": f(inputs["proj_b"]),
        "bias_pp": np.ascontiguousarray(bias_pp, dtype=np.float32),
    }


def make_in_maps(inputs):
    full = host_fold(inputs)
    in_maps = []
    for c in range(NC):
        m = dict(full)
        m["x"] = np.ascontiguousarray(full["x"][c * TPC:(c + 1) * TPC])
        in_maps.append(m)
    return in_maps


def kernel(**inputs) -> np.ndarray:
    from concourse import bass_utils
    nc = _get_program()
    in_maps = make_in_maps(inputs)
    res = bass_utils.run_bass_kernel_spmd(nc, in_maps, core_ids=list(range(NC)))
    out = np.concatenate([res.results[c]["out"] for c in range(NC)], axis=0)
    return out.reshape(B, S, D)
